# revision 14
# baseline (speedup 1.0000x reference)
"""Trainium2 Bass kernel for BiDecoder edge dot products.

out[e] = dot(ufeat[src[e]], ifeat[dst[e]])   for E=300000 edges, D=256.

Strategy (8 NeuronCores, SPMD):
  - Shard edges across the 8 cores (37500 each); replicate the node tables.
  - Per core, sort edges by dst. dma_gather needs int16 row indices, so the
    ifeat table (50000 rows) is addressed as two bases: rows [0, 32768) and
    rows [32768, 50000). Sorting by dst makes each group contiguous; groups
    are padded to a whole number of gather chunks (caps taken as the max
    across cores so all cores run one identical program).
  - On device: chunked dma_gather (SWDGE gather, 1024 rows x 1KB per call)
    of hu and hv, double buffered; DVE tensor_tensor_reduce fuses the
    per-edge multiply + row-sum; one final DMA writes all dots out.
  - Host reorders the per-slot outputs back to original edge order.
"""

import sys

for _p in ("/opt/trn_rl_repo",):
    if _p not in sys.path:
        sys.path.append(_p)

import numpy as np

P = 128
D = 256
E = 300000
NCORES = 8
ECORE = E // NCORES
N_GENE = 20000
N_CELL = 50000
SPLIT = 32768            # ifeat rows below/above this use different gather bases
C_TILE = 8               # tiles (of 128 edges) per gather chunk
CHUNK_E = C_TILE * P     # 1024 edges per dma_gather call
COLS = CHUNK_E // 16     # idx columns per chunk in the wrapped layout

_PROGRAM_CACHE: dict = {}


def _cdiv(a, b):
    return -(-a // b)


def _wrap_idx(idx_i16: np.ndarray, nchunk: int) -> np.ndarray:
    """[nchunk*CHUNK_E] int16 -> [128, nchunk*COLS] dma_gather idx layout.

    Within each chunk, index i lives at partition i%16, column i//16; the
    16-partition block is replicated 8x down the 128 partitions.
    """
    w = idx_i16.reshape(nchunk, COLS, 16).transpose(2, 0, 1).reshape(16, nchunk * COLS)
    return np.ascontiguousarray(np.tile(w, (8, 1)))


def _build_program(na: int, nb: int, n_gene: int = N_GENE, n_cell: int = N_CELL,
                   split: int = SPLIT):
    import concourse.bacc as bacc
    import concourse.mybir as mybir
    from concourse.library_config import mlp

    nchunk = na + nb
    ntiles = nchunk * C_TILE
    totcols = nchunk * COLS

    NSLOT = 6  # buffer slots (chunk c uses slot c % NSLOT); SWDGE queues fixed at 4

    nc = bacc.Bacc("TRN2", debug=False, num_swdge_queues=4,
                   dynamic_dma_scratch_size=65536)
    ufeat = nc.dram_tensor("ufeat", [n_gene, D], mybir.dt.float32, kind="ExternalInput")
    ifeat = nc.dram_tensor("ifeat", [n_cell, D], mybir.dt.float32, kind="ExternalInput")
    sidx = nc.dram_tensor("sidx", [P, totcols], mybir.dt.int16, kind="ExternalInput")
    didx = nc.dram_tensor("didx", [P, totcols], mybir.dt.int16, kind="ExternalInput")
    y = nc.dram_tensor("y", [P, ntiles], mybir.dt.float32, kind="ExternalOutput")

    with (
        nc.sbuf_tensor("hu", [P, NSLOT, C_TILE, D], mybir.dt.float32) as hu,
        nc.sbuf_tensor("hv", [P, NSLOT, C_TILE, D], mybir.dt.float32) as hv,
        nc.sbuf_tensor("sidx_sb", [P, totcols], mybir.dt.int16) as sidx_sb,
        nc.sbuf_tensor("didx_sb", [P, totcols], mybir.dt.int16) as didx_sb,
        nc.sbuf_tensor("osb", [P, ntiles], mybir.dt.float32) as osb,
        nc.semaphore("io") as io,
        nc.semaphore("cons") as cons,
        nc.semaphore("io2") as io2,
        nc.Block() as block,
        __import__("contextlib").ExitStack() as _stk,
    ):
        gu = [_stk.enter_context(nc.semaphore(f"gu{i}")) for i in range(NSLOT)]
        gv = [_stk.enter_context(nc.semaphore(f"gv{i}")) for i in range(NSLOT)]

        @block.gpsimd
        def _(gp):
            gp.load_library(mlp)
            gp.wait_ge(io, 32)
            for c in range(nchunk):
                s = c % NSLOT
                if c >= NSLOT:
                    gp.wait_ge(cons, c - NSLOT + 1)
                cols = slice(c * COLS, (c + 1) * COLS)
                gp.dma_gather(
                    hu[:, s], ufeat[:, :], sidx_sb[:, cols], CHUNK_E, CHUNK_E, D,
                    queue_num=(2 * c) % 4,
                ).then_inc(gu[s], 16)
                base = ifeat[:split, :] if c < na else ifeat[split:, :]
                gp.dma_gather(
                    hv[:, s], base, didx_sb[:, cols], CHUNK_E, CHUNK_E, D,
                    queue_num=(2 * c + 1) % 4,
                ).then_inc(gv[s], 16)
            for s in range(NSLOT):
                cnt = (nchunk - s + NSLOT - 1) // NSLOT
                if cnt:
                    gp.wait_ge(gu[s], 16 * cnt)
                    gp.wait_ge(gv[s], 16 * cnt)

        @block.vector
        def _(v):
            for c in range(nchunk):
                s = c % NSLOT
                k = c // NSLOT + 1
                v.wait_ge(gu[s], 16 * k)
                v.wait_ge(gv[s], 16 * k)
                for t in range(C_TILE):
                    col = c * C_TILE + t
                    inst = v.affine_mul_reduce(
                        out=hv[:, s, t, :],
                        accum_out=osb[:, col : col + 1],
                        in0=hu[:, s, t, :],
                        in1=hv[:, s, t, :],
                        scale=1.0,
                        bias=0.0,
                    )
                    if t == C_TILE - 1:
                        inst.then_inc(cons, 1)

        @block.sync
        def _(sy):
            sy.dma_start(sidx_sb[:], sidx[:]).then_inc(io, 16)
            sy.dma_start(didx_sb[:], didx[:]).then_inc(io, 16)
            sy.wait_ge(cons, nchunk)
            sy.dma_start(y[:, :], osb[:, :]).then_inc(io2, 16)
            sy.wait_ge(io2, 16)

    nc.compile()
    return nc


def _prep_core(s_j, d_j, ids_j, na, nb):
    """Build one core's slot arrays: wrapped int16 idx tensors + edge ids."""
    nslot = (na + nb) * CHUNK_E
    a = int((d_j < SPLIT).sum())
    sidx = np.zeros(nslot, np.int16)
    didx = np.zeros(nslot, np.int16)
    eid = np.full(nslot, -1, np.int64)
    sidx[:a] = s_j[:a].astype(np.int16)
    didx[:a] = d_j[:a].astype(np.int16)
    eid[:a] = ids_j[:a]
    boff = na * CHUNK_E
    nbj = len(d_j) - a
    sidx[boff : boff + nbj] = s_j[a:].astype(np.int16)
    didx[boff : boff + nbj] = (d_j[a:] - SPLIT).astype(np.int16)
    eid[boff : boff + nbj] = ids_j[a:]
    return (
        _wrap_idx(sidx, na + nb),
        _wrap_idx(didx, na + nb),
        eid,
    )


def kernel(ufeat, ifeat, src, dst):
    from concourse.bass_utils import run_bass_kernel_spmd

    ufeat = np.ascontiguousarray(np.asarray(ufeat), dtype=np.float32)
    ifeat = np.ascontiguousarray(np.asarray(ifeat), dtype=np.float32)
    src_f = np.asarray(src).ravel().astype(np.int64)
    dst_f = np.asarray(dst).ravel().astype(np.int64)
    assert src_f.shape == (E,) and dst_f.shape == (E,)

    cores = []
    for j in range(NCORES):
        lo, hi = j * ECORE, (j + 1) * ECORE
        d_j = dst_f[lo:hi]
        order = np.argsort(d_j, kind="stable")
        cores.append((src_f[lo:hi][order], d_j[order], np.arange(lo, hi)[order]))

    n_a = [int((d < SPLIT).sum()) for (_, d, _) in cores]
    na = max(1, max(_cdiv(a, CHUNK_E) for a in n_a))
    nb = max(1, max(_cdiv(ECORE - a, CHUNK_E) for a in n_a))

    key = (na, nb)
    if key not in _PROGRAM_CACHE:
        _PROGRAM_CACHE[key] = _build_program(na, nb)
    nc = _PROGRAM_CACHE[key]

    in_maps = []
    eids = []
    for j in range(NCORES):
        s_j, d_j, ids_j = cores[j]
        sidx_w, didx_w, eid = _prep_core(s_j, d_j, ids_j, na, nb)
        in_maps.append({"ufeat": ufeat, "ifeat": ifeat, "sidx": sidx_w, "didx": didx_w})
        eids.append(eid)

    res = run_bass_kernel_spmd(nc, in_maps, core_ids=list(range(NCORES)))

    out = np.empty((E, 1), np.float32)
    for j in range(NCORES):
        yj = res.results[j]["y"]          # [128, ntiles]; slot i -> y[i%128, i//128]
        vals = np.ascontiguousarray(yj.T).ravel()
        m = eids[j] >= 0
        out[eids[j][m], 0] = vals[m]
    return out
